# revision 1
# baseline (speedup 1.0000x reference)
"""LoRALinear Trainium2 kernel.

out = x @ W^T + bias + 2.0 * ((x @ A^T) @ B^T)

Strategy (v4):
  - 2x4 sharding over 8 NeuronCores: M split 2-way, out_features 4-way.
    Per core: x-shard [4096 tok, 4096 k] (32Mi), W-shard [1024 out, 4096 k]
    (16Mi). The small W-shard keeps the W-stream window short enough to hide
    under k-outer head absorption; x streams per m-tile-pair all kernel long.
  - Host ships k-major (pre-transposed) layouts - pure data layout only.
  - LoRA folded on-chip: Weff^T[kt] = W^T[kt] + (2*B@A)^T[kt]. The rank-16
    matmul (BA) lands in PSUM and the fold rides the W f32->f16 cast as a
    single DVE tensor_add(psum, w32) -> f16. Bias is added at eviction.
  - Mixed precision: k-tiles 0..5 as fp8e4 DoubleRow pairs (256-deep
    contraction, 2x FLOP rate), k-tiles 6..31 fp16. fp8 operands scaled
    W*8 / x*(1/8) (exact pow2) to pull W out of fp8-subnormal range.
    Measured (numpy, exact inputs): rel_max ~1.6e-2 vs the 2e-2 gate.
  - Head: while W streams, BA(kt) + 3 m-tiles x 2 o-chunks absorb each
    arriving W k-tile (k-outer), fp8 k-range first in the queue.
"""

import numpy as np

IN_F = 4096
OUT_F = 4096
R = 16
SCALING = 2.0
M = 4 * 2048  # 8192 tokens

N_CORES = 8
M_SPLIT = 2
O_SPLIT = 4
M_SH = M // M_SPLIT      # 4096 rows per core
O_SH = OUT_F // O_SPLIT  # 1024 out-features per core
K = IN_F
KT = K // 128            # 32 k-tiles
N_KP8 = 3                # 3 fp8 k-pairs = k-tiles 0..5
KT16_0 = 2 * N_KP8       # first f16 k-tile (6)
N_MT = M_SH // 128       # 32 m-tiles
N_PAIR = N_MT // 2       # 16 m-tile pairs (x fetched per pair)
WS8 = 8.0                # fp8 scaling: W*8, x/8

_NC_CACHE = {}
LAST_RESULT = None


def _build():
    import concourse.mybir as mybir
    import concourse.tile as tile
    from concourse import bacc

    f32, f16, f8 = mybir.dt.float32, mybir.dt.float16, mybir.dt.float8e4
    DR = mybir.MatmulPerfMode.DoubleRow

    nc = bacc.Bacc(
        "TRN2", target_bir_lowering=False, debug=False, num_devices=N_CORES
    )
    xt_d = nc.dram_tensor("xt", [K, M_SH], f32, kind="ExternalInput")
    wt_d = nc.dram_tensor("wt", [K, O_SH], f32, kind="ExternalInput")
    a_d = nc.dram_tensor("a", [R, K], f32, kind="ExternalInput")
    bt_d = nc.dram_tensor("bt", [R, O_SH], f32, kind="ExternalInput")
    bias_d = nc.dram_tensor("bias", [O_SH], f32, kind="ExternalInput")
    out_d = nc.dram_tensor("out", [M_SH, O_SH], f32, kind="ExternalOutput")

    with tile.TileContext(nc) as tc:
        with (
            tc.tile_pool(name="const", bufs=1) as const,
            tc.tile_pool(name="wfp", bufs=1) as wfp,
            tc.tile_pool(name="wp8p", bufs=1) as wp8p,
            tc.tile_pool(name="w32p", bufs=4) as w32p,
            tc.tile_pool(name="xfp", bufs=3) as xfp,
            tc.tile_pool(name="xpp", bufs=3) as xpp,
            tc.tile_pool(name="outp", bufs=3) as outp,
        ):
            # ---- constants ----
            junk = const.tile([128, 512], f16)
            nc.vector.memset(junk[:], 0.0)
            a_sb = const.tile([R, K], f16)
            nc.gpsimd.dma_start(a_sb[:], a_d[:])
            btr = const.tile([R, O_SH], f16)
            nc.gpsimd.dma_start(btr[:], bt_d[:])
            bt2 = const.tile([R, O_SH], f16)  # 2 * B^T
            nc.vector.tensor_scalar_mul(bt2[:], btr[:], SCALING)
            bias_bc = const.tile([128, O_SH], f32)
            nc.gpsimd.dma_start(bias_bc[:], bias_d[:].partition_broadcast(128))

            wf = {}       # f16 folded W tiles  [128, O_SH]
            wp8 = [wp8p.tile([128, 2, O_SH], f8, name=f"wp8_{kp}")
                   for kp in range(N_KP8)]

            def w_dma(kt):
                w32 = w32p.tile([128, O_SH], f32, name="w32")
                eng = nc.sync if kt % 2 == 0 else nc.scalar
                eng.dma_start(w32[:], wt_d[kt * 128 : (kt + 1) * 128, :])
                return w32

            def ba_fold(kt, pba_pool, w32):
                """BA matmul + fold into wf[kt] (f16)."""
                ks = slice(kt * 128, (kt + 1) * 128)
                wft = wfp.tile([128, O_SH], f16, name=f"wf{kt}")
                for h in range(2):
                    hs = slice(h * 512, (h + 1) * 512)
                    pba = pba_pool.tile([128, 512], f32, name=f"pba{h}")
                    nc.tensor.matmul(
                        pba[:], a_sb[:, ks], bt2[:, hs], start=True, stop=True
                    )
                    nc.vector.tensor_add(wft[:, hs], pba[:], w32[:, hs])
                wf[kt] = wft

            xfs = [None] * N_PAIR  # f16 x tiles [128, KT, 256] per m-pair
            xps = [None] * N_PAIR  # fp8 paired x tiles [128, N_KP8, 2, 256]

            def fetch_x(pr):
                ms = slice(pr * 256, (pr + 1) * 256)
                xf = xfp.tile([128, KT, 256], f16, name="xf")
                nc.gpsimd.dma_start(
                    xf[:], xt_d[:, ms].rearrange("(kt p) m -> p kt m", p=128)
                )
                xp = xpp.tile([128, N_KP8, 2, 256], f8, name="xp")
                for kp in range(N_KP8):
                    for i in range(2):
                        nc.scalar.activation(
                            xp[:, kp, i, :], xf[:, 2 * kp + i, :],
                            mybir.ActivationFunctionType.Copy, scale=1.0 / WS8,
                        )
                xfs[pr], xps[pr] = xf, xp

            def xslice(mt, kt):
                pr, half = divmod(mt, 2)
                return xfs[pr][:, kt, half * 128 : half * 128 + 128]

            def xslice8(mt, kp):
                pr, half = divmod(mt, 2)
                return xps[pr][:, kp, :, half * 128 : half * 128 + 128]

            def f16_pass(mt, oc, pm, kt, start=False):
                nc.tensor.matmul(
                    pm[:], xslice(mt, kt), wf[kt][:, oc * 512 : (oc + 1) * 512],
                    start=start, stop=False,
                )

            def fp8_passes(mt, oc, pm):
                # never start=True: HW PSUM reset is bank-wide; a 256-col DR
                # pass with start would wipe its sibling half
                for kp in range(N_KP8):
                    for h in range(2):
                        cs = slice(oc * 512 + h * 256, oc * 512 + (h + 1) * 256)
                        nc.tensor.matmul(
                            pm[:, h * 256 : (h + 1) * 256],
                            xslice8(mt, kp), wp8[kp][:, :, cs],
                            start=False, stop=False, perf_mode=DR,
                        )

            def close_group(mt, oc, pm, osb):
                # last f16 k-tile carries stop
                nc.tensor.matmul(
                    pm[:], xslice(mt, KT - 1),
                    wf[KT - 1][:, oc * 512 : (oc + 1) * 512],
                    start=False, stop=True,
                )
                hs = slice(oc * 512, (oc + 1) * 512)
                nc.vector.tensor_add(osb[:, hs], pm[:], bias_bc[:, hs])

            HEAD_MTS = (0, 1, 2)

            with tc.tile_pool(name="bap", bufs=1, space="PSUM") as bap, \
                 tc.tile_pool(name="hps", bufs=1, space="PSUM") as hps:
                # warmup junk burst flips the PE clock gate early; uses a BA
                # ring slot before the first real BA pass
                pwarm = bap.tile([128, 512], f32, name="pba0")
                for _ in range(24):
                    nc.tensor.matmul(
                        pwarm[:], junk[:, 0:128], junk[:], start=True, stop=True
                    )

                # fp8 k-range first: BA + fold + fp8 quantize for kts 0..5
                for kt in range(KT16_0):
                    ba_fold(kt, bap, w_dma(kt))
                for kp in range(N_KP8):
                    for i in range(2):
                        nc.scalar.activation(
                            wp8[kp][:, i, :], wf[2 * kp + i][:],
                            mybir.ActivationFunctionType.Copy, scale=WS8,
                        )

                fetch_x(0)
                fetch_x(1)

                hpm = {
                    (mt, oc): hps.tile([128, 512], f32, name=f"h{mt}_{oc}")
                    for mt in HEAD_MTS for oc in range(2)
                }
                # open groups with a full-width f16 pass on kt6
                w32 = w_dma(KT16_0)
                ba_fold(KT16_0, bap, w32)
                for mt in HEAD_MTS:
                    for oc in range(2):
                        f16_pass(mt, oc, hpm[(mt, oc)], KT16_0, start=True)
                for mt in HEAD_MTS:
                    for oc in range(2):
                        fp8_passes(mt, oc, hpm[(mt, oc)])
                # k-outer absorption: BA(kt) + 6 head passes per arriving kt
                for kt in range(KT16_0 + 1, KT):
                    ba_fold(kt, bap, w_dma(kt))
                    if kt < KT - 1:
                        for mt in HEAD_MTS:
                            for oc in range(2):
                                f16_pass(mt, oc, hpm[(mt, oc)], kt)
                osbh = {}
                for mt in HEAD_MTS:
                    osbh[mt] = outp.tile([128, O_SH], f32, name="osb")
                    for oc in range(2):
                        close_group(mt, oc, hpm[(mt, oc)], osbh[mt])
                    nc.scalar.dma_start(
                        out_d[mt * 128 : (mt + 1) * 128, :], osbh[mt][:]
                    )

            # ---- steady phase ----
            with tc.tile_pool(name="sps", bufs=3, space="PSUM") as sps:
                fetch_x(2)
                for mt in range(len(HEAD_MTS), N_MT):
                    pr = mt // 2
                    if mt % 2 == 0 and pr + 1 < N_PAIR:
                        fetch_x(pr + 1)
                    osb = outp.tile([128, O_SH], f32, name="osb")
                    for oc in range(2):
                        pm = sps.tile([128, 512], f32, name="pm")
                        f16_pass(mt, oc, pm, KT16_0, start=True)
                        fp8_passes(mt, oc, pm)
                        for kt in range(KT16_0 + 1, KT - 1):
                            f16_pass(mt, oc, pm, kt)
                        close_group(mt, oc, pm, osb)
                    nc.scalar.dma_start(
                        out_d[mt * 128 : (mt + 1) * 128, :], osb[:]
                    )

    nc.compile()
    return nc


def _get_nc():
    if "nc" not in _NC_CACHE:
        _NC_CACHE["nc"] = _build()
    return _NC_CACHE["nc"]


def kernel(x, weight, bias, A, B):
    global LAST_RESULT
    from concourse.bass_utils import run_bass_kernel_spmd

    x = np.asarray(x, dtype=np.float32).reshape(M, K)
    weight = np.asarray(weight, dtype=np.float32)
    bias = np.asarray(bias, dtype=np.float32)
    A = np.ascontiguousarray(np.asarray(A, dtype=np.float32))
    B = np.asarray(B, dtype=np.float32)

    # Host-side layout prep (transposes only; no arithmetic).
    xt_slabs = [
        np.ascontiguousarray(x[mi * M_SH : (mi + 1) * M_SH].T)
        for mi in range(M_SPLIT)
    ]
    wt_slabs, bt_slabs, bias_slabs = [], [], []
    for oi in range(O_SPLIT):
        os_ = slice(oi * O_SH, (oi + 1) * O_SH)
        wt_slabs.append(np.ascontiguousarray(weight[os_].T))
        bt_slabs.append(np.ascontiguousarray(B[os_].T))
        bias_slabs.append(np.ascontiguousarray(bias[os_]))

    nc = _get_nc()
    in_maps = []
    for c in range(N_CORES):
        mi, oi = divmod(c, O_SPLIT)
        in_maps.append(
            {
                "xt": xt_slabs[mi],
                "wt": wt_slabs[oi],
                "a": A,
                "bt": bt_slabs[oi],
                "bias": bias_slabs[oi],
            }
        )

    res = run_bass_kernel_spmd(nc, in_maps, list(range(N_CORES)))
    LAST_RESULT = res

    out = np.empty((M, OUT_F), np.float32)
    for c in range(N_CORES):
        mi, oi = divmod(c, O_SPLIT)
        out[mi * M_SH : (mi + 1) * M_SH, oi * O_SH : (oi + 1) * O_SH] = (
            res.results[c]["out"]
        )
    return out.reshape(4, 2048, OUT_F)



# revision 3
# speedup vs baseline: 1.2814x; 1.2814x over previous
"""LoRALinear Trainium2 kernel.

out = x @ W^T + bias + 2.0 * ((x @ A^T) @ B^T)

Strategy (v5):
  - 2x4 sharding over 8 NeuronCores: M split 2-way, out_features 4-way.
    Per core: x-shard [4096 tok, 4096 k], W-shard [1024 out, 4096 k].
  - Host ships k-major pre-tiled f16 layouts (layout + dtype cast only):
    x as [16 pair, 128 p, 32 kt, 256 m] f16 (contiguous 2MiB per pair
    fetch), W as [128 p, 32 kt, 1024 o] f16 (contiguous 2MiB per 8-kt
    chunk). All big DMAs are HWDGE line-rate contiguous.
  - LoRA folded on-chip: wf[kt] = W16[kt] + (2*B@A)^T[kt] via rank-16 PE
    matmul (BA) + DVE add. Bias added at eviction.
  - Mixed precision: k-tiles 0..7 as fp8e4 DoubleRow pairs (4 DR passes
    of FD=512, each covering 2 k-tiles), k-tiles 8..31 f16 (24 passes of
    FD=512). fp8 operands scaled W*8 / x*(1/8) (exact pow2). ACT-engine
    f32->f16->f8 conversions are bit-exact RNE (probe-verified).
    Numpy-sim rel_max vs cpu reference: 1.872e-2 (gate 2e-2).
  - Group order: DR kp0 (start=True, full-bank FD=512) .. kp3, then f16
    kt8..kt30, kt31 carries stop; DVE adds bias at eviction.
  - Head: 3 m-tiles x 2 oc absorb the W-chunk stream k-outer while BA
    folds ride chunk arrivals; steady phase is k-inner per m-tile.
"""

import numpy as np

IN_F = 4096
OUT_F = 4096
R = 16
SCALING = 2.0
M = 4 * 2048  # 8192 tokens

N_CORES = 8
M_SPLIT = 2
O_SPLIT = 4
M_SH = M // M_SPLIT      # 4096 rows per core
O_SH = OUT_F // O_SPLIT  # 1024 out-features per core
K = IN_F
KT = K // 128            # 32 k-tiles
N_KP = 4                 # fp8 k-pairs = k-tiles 0..7
KT16_0 = 2 * N_KP        # first f16 k-tile (8)
N_MT = M_SH // 128       # 32 m-tiles
N_PAIR = N_MT // 2       # 16 m-tile pairs (x fetched per pair)
N_WCH = 4                # W DMA chunks (8 k-tiles each)
WS8 = 8.0                # fp8 scaling: W*8, x/8

_NC_CACHE = {}
LAST_RESULT = None


def _build():
    import concourse.mybir as mybir
    import concourse.tile as tile
    from concourse import bacc

    f32, f16, f8 = mybir.dt.float32, mybir.dt.float16, mybir.dt.float8e4
    DR = mybir.MatmulPerfMode.DoubleRow
    COPY = mybir.ActivationFunctionType.Copy

    nc = bacc.Bacc(
        "TRN2", target_bir_lowering=False, debug=False, num_devices=N_CORES
    )
    xtl_d = nc.dram_tensor("xtl", [N_PAIR, 128, KT, 256], f16,
                           kind="ExternalInput")
    wtl_d = nc.dram_tensor("wtl", [128, KT, O_SH], f16, kind="ExternalInput")
    a_d = nc.dram_tensor("a", [R, K], f16, kind="ExternalInput")
    bt_d = nc.dram_tensor("bt", [R, O_SH], f16, kind="ExternalInput")
    bias_d = nc.dram_tensor("bias", [O_SH], f32, kind="ExternalInput")
    out_d = nc.dram_tensor("out", [M_SH, O_SH], f32, kind="ExternalOutput")

    with tile.TileContext(nc) as tc:
        with (
            tc.tile_pool(name="const", bufs=1) as const,
            tc.tile_pool(name="wfp", bufs=1) as wfp,
            tc.tile_pool(name="wsp", bufs=2) as wsp,
            tc.tile_pool(name="wp8p", bufs=1) as wp8p,
            tc.tile_pool(name="xfp", bufs=3) as xfp,
            tc.tile_pool(name="xpp", bufs=3) as xpp,
            tc.tile_pool(name="outp", bufs=3) as outp,
        ):
            # ---- constants ----
            junk = const.tile([128, 512], f16)
            nc.vector.memset(junk[:], 0.0)
            a_sb = const.tile([R, K], f16)
            nc.gpsimd.dma_start(a_sb[:], a_d[:])
            bt_sb = const.tile([R, O_SH], f16)
            nc.gpsimd.dma_start(bt_sb[:], bt_d[:])
            bt2 = const.tile([R, O_SH], f16)  # 2 * B^T
            nc.vector.tensor_scalar_mul(bt2[:], bt_sb[:], SCALING)
            bias_bc = const.tile([128, O_SH], f32)
            nc.gpsimd.dma_start(bias_bc[:], bias_d[:].partition_broadcast(128))

            wf = {}       # folded W tiles, f16 [128, O_SH]
            wp8 = [wp8p.tile([128, 2, O_SH], f8, name=f"wp8_{kp}")
                   for kp in range(N_KP)]

            def ba_fold(kt, pba_pool, w16):
                """BA matmul + fold (w16 + 2BA^T) -> wf[kt] (f16)."""
                ks = slice(kt * 128, (kt + 1) * 128)
                wft = wfp.tile([128, O_SH], f16, name=f"wf{kt}")
                for h in range(2):
                    hs = slice(h * 512, (h + 1) * 512)
                    pba = pba_pool.tile([128, 512], f32, name=f"pba{h}")
                    nc.tensor.matmul(
                        pba[:], a_sb[:, ks], bt2[:, hs], start=True, stop=True
                    )
                    nc.vector.tensor_add(wft[:, hs], pba[:], w16[:, hs])
                wf[kt] = wft

            xfs = [None] * N_PAIR  # f16 x tiles [128, KT, 256] per m-pair
            xps = [None] * N_PAIR  # fp8 paired x tiles [128, N_KP, 2, 256]

            def fetch_x(pr):
                xf = xfp.tile([128, KT, 256], f16, name="xf")
                nc.scalar.dma_start(xf[:], xtl_d[pr, :, :, :])
                xp = xpp.tile([128, N_KP, 2, 256], f8, name="xp")
                for kp in range(N_KP):
                    for i in range(2):
                        nc.scalar.activation(
                            xp[:, kp, i, :], xf[:, 2 * kp + i, :],
                            COPY, scale=1.0 / WS8,
                        )
                xfs[pr], xps[pr] = xf, xp

            def xslice(mt, kt):
                pr, half = divmod(mt, 2)
                return xfs[pr][:, kt, half * 128 : half * 128 + 128]

            def xslice8(mt, kp):
                pr, half = divmod(mt, 2)
                return xps[pr][:, kp, :, half * 128 : half * 128 + 128]

            def dr_pass(mt, oc, pm, kp, start=False):
                ocs = slice(oc * 512, (oc + 1) * 512)
                nc.tensor.matmul(
                    pm[:], xslice8(mt, kp), wp8[kp][:, :, ocs],
                    start=start, stop=False, perf_mode=DR,
                )

            def f16_pass(mt, oc, pm, kt):
                nc.tensor.matmul(
                    pm[:], xslice(mt, kt), wf[kt][:, oc * 512 : (oc + 1) * 512],
                    start=False, stop=False,
                )

            def close_group(mt, oc, pm, osb):
                nc.tensor.matmul(
                    pm[:], xslice(mt, KT - 1),
                    wf[KT - 1][:, oc * 512 : (oc + 1) * 512],
                    start=False, stop=True,
                )
                hs = slice(oc * 512, (oc + 1) * 512)
                nc.vector.tensor_add(osb[:, hs], pm[:], bias_bc[:, hs])

            HEAD_MTS = (0, 1, 2)

            with tc.tile_pool(name="bap", bufs=1, space="PSUM") as bap, \
                 tc.tile_pool(name="hps", bufs=1, space="PSUM") as hps:
                # warmup burst flips the PE clock gate early
                pwarm = bap.tile([128, 512], f32, name="pba0")
                for _ in range(8):
                    nc.tensor.matmul(
                        pwarm[:], junk[:, 0:128], junk[:], start=True, stop=True
                    )

                fetch_x(0)
                fetch_x(1)

                hpm = {
                    (mt, oc): hps.tile([128, 512], f32, name=f"h{mt}_{oc}")
                    for mt in HEAD_MTS for oc in range(2)
                }

                # W chunk stream: BA+fold per kt; chunk 0 also feeds the fp8
                # quantize; chunks 1..3 are absorbed k-outer by head groups.
                for c in range(N_WCH):
                    w16c = wsp.tile([128, 8, O_SH], f16, name="w16c")
                    nc.sync.dma_start(w16c[:], wtl_d[:, c * 8 : (c + 1) * 8, :])
                    for j in range(8):
                        ba_fold(c * 8 + j, bap, w16c[:, j, :])
                    if c == 0:
                        for kp in range(N_KP):
                            for i in range(2):
                                nc.scalar.activation(
                                    wp8[kp][:, i, :], wf[2 * kp + i][:],
                                    COPY, scale=WS8,
                                )
                        # open head groups with fp8 DR passes
                        for mt in HEAD_MTS:
                            for oc in range(2):
                                dr_pass(mt, oc, hpm[(mt, oc)], 0, start=True)
                        for kp in range(1, N_KP):
                            for mt in HEAD_MTS:
                                for oc in range(2):
                                    dr_pass(mt, oc, hpm[(mt, oc)], kp)
                    else:
                        for j in range(8):
                            kt = c * 8 + j
                            if kt == KT - 1:
                                continue  # kt31 carried by close_group
                            for mt in HEAD_MTS:
                                for oc in range(2):
                                    f16_pass(mt, oc, hpm[(mt, oc)], kt)

                fetch_x(2)
                osbh = {}
                for mt in HEAD_MTS:
                    osbh[mt] = outp.tile([128, O_SH], f32, name="osb")
                    for oc in range(2):
                        close_group(mt, oc, hpm[(mt, oc)], osbh[mt])
                    nc.sync.dma_start(
                        out_d[mt * 128 : (mt + 1) * 128, :], osbh[mt][:]
                    )

            # ---- steady phase ----
            with tc.tile_pool(name="sps", bufs=4, space="PSUM") as sps:
                for mt in range(len(HEAD_MTS), N_MT):
                    pr = mt // 2
                    if mt % 2 == 0 and pr + 1 < N_PAIR:
                        fetch_x(pr + 1)
                    osb = outp.tile([128, O_SH], f32, name="osb")
                    for oc in range(2):
                        pm = sps.tile([128, 512], f32, name="pm")
                        dr_pass(mt, oc, pm, 0, start=True)
                        for kp in range(1, N_KP):
                            dr_pass(mt, oc, pm, kp)
                        for kt in range(KT16_0, KT - 1):
                            f16_pass(mt, oc, pm, kt)
                        close_group(mt, oc, pm, osb)
                    nc.sync.dma_start(
                        out_d[mt * 128 : (mt + 1) * 128, :], osb[:]
                    )

    nc.compile()
    return nc


def _get_nc():
    if "nc" not in _NC_CACHE:
        _NC_CACHE["nc"] = _build()
    return _NC_CACHE["nc"]


def kernel(x, weight, bias, A, B):
    global LAST_RESULT
    from concourse.bass_utils import run_bass_kernel_spmd

    x = np.asarray(x, dtype=np.float32).reshape(M, K)
    weight = np.asarray(weight, dtype=np.float32)
    bias = np.asarray(bias, dtype=np.float32)
    A = np.asarray(A, dtype=np.float32)
    B = np.asarray(B, dtype=np.float32)

    # Host-side layout prep (transposes + f16 casts only; no arithmetic).
    xtl_slabs = []
    for mi in range(M_SPLIT):
        xt = x[mi * M_SH : (mi + 1) * M_SH].T  # [K, M_SH]
        v = xt.reshape(KT, 128, N_PAIR, 256).transpose(2, 1, 0, 3)
        xtl_slabs.append(np.ascontiguousarray(v, dtype=np.float16))
    wtl_slabs, bt_slabs, bias_slabs = [], [], []
    for oi in range(O_SPLIT):
        os_ = slice(oi * O_SH, (oi + 1) * O_SH)
        wt = weight[os_].T  # [K, O_SH]
        v = wt.reshape(KT, 128, O_SH).transpose(1, 0, 2)
        wtl_slabs.append(np.ascontiguousarray(v, dtype=np.float16))
        bt_slabs.append(np.ascontiguousarray(B[os_].T, dtype=np.float16))
        bias_slabs.append(np.ascontiguousarray(bias[os_]))
    a_f16 = np.ascontiguousarray(A, dtype=np.float16)

    nc = _get_nc()
    in_maps = []
    for c in range(N_CORES):
        mi, oi = divmod(c, O_SPLIT)
        in_maps.append(
            {
                "xtl": xtl_slabs[mi],
                "wtl": wtl_slabs[oi],
                "a": a_f16,
                "bt": bt_slabs[oi],
                "bias": bias_slabs[oi],
            }
        )

    res = run_bass_kernel_spmd(nc, in_maps, list(range(N_CORES)))
    LAST_RESULT = res

    out = np.empty((M, OUT_F), np.float32)
    for c in range(N_CORES):
        mi, oi = divmod(c, O_SPLIT)
        out[mi * M_SH : (mi + 1) * M_SH, oi * O_SH : (oi + 1) * O_SH] = (
            res.results[c]["out"]
        )
    return out.reshape(4, 2048, OUT_F)


# revision 5
# speedup vs baseline: 1.2921x; 1.0083x over previous
"""LoRALinear Trainium2 kernel.

out = x @ W^T + bias + 2.0 * ((x @ A^T) @ B^T)

Strategy (v5):
  - 2x4 sharding over 8 NeuronCores: M split 2-way, out_features 4-way.
    Per core: x-shard [4096 tok, 4096 k], W-shard [1024 out, 4096 k].
  - Host ships k-major pre-tiled f16 layouts (layout + dtype cast only):
    x as [16 pair, 128 p, 32 kt, 256 m] f16 (contiguous 2MiB per pair
    fetch), W as [128 p, 32 kt, 1024 o] f16 (contiguous 2MiB per 8-kt
    chunk). All big DMAs are HWDGE line-rate contiguous.
  - LoRA folded on-chip: wf[kt] = W16[kt] + (2*B@A)^T[kt] via rank-16 PE
    matmul (BA) + DVE add. Bias added at eviction.
  - Mixed precision: k-tiles 0..7 as fp8e4 DoubleRow pairs (4 DR passes
    of FD=512, each covering 2 k-tiles), k-tiles 8..31 f16 (24 passes of
    FD=512). fp8 operands scaled W*8 / x*(1/8) (exact pow2). ACT-engine
    f32->f16->f8 conversions are bit-exact RNE (probe-verified).
    Numpy-sim rel_max vs cpu reference: 1.872e-2 (gate 2e-2).
  - Group order: DR kp0 (start=True, full-bank FD=512) .. kp3, then f16
    kt8..kt30, kt31 carries stop; DVE adds bias at eviction.
  - Head: 3 m-tiles x 2 oc absorb the W-chunk stream k-outer while BA
    folds ride chunk arrivals; steady phase is k-inner per m-tile.
"""

import numpy as np

IN_F = 4096
OUT_F = 4096
R = 16
SCALING = 2.0
M = 4 * 2048  # 8192 tokens

N_CORES = 8
M_SPLIT = 2
O_SPLIT = 4
M_SH = M // M_SPLIT      # 4096 rows per core
O_SH = OUT_F // O_SPLIT  # 1024 out-features per core
K = IN_F
KT = K // 128            # 32 k-tiles
N_KP = 4                 # fp8 k-pairs = k-tiles 0..7
KT16_0 = 2 * N_KP        # first f16 k-tile (8)
N_MT = M_SH // 128       # 32 m-tiles
N_PAIR = N_MT // 2       # 16 m-tile pairs (x fetched per pair)
N_WCH = 4                # W DMA chunks (8 k-tiles each)
WS8 = 8.0                # fp8 scaling: W*8, x/8

_NC_CACHE = {}
LAST_RESULT = None


def _build():
    import concourse.mybir as mybir
    import concourse.tile as tile
    from concourse import bacc

    f32, f16, f8 = mybir.dt.float32, mybir.dt.float16, mybir.dt.float8e4
    DR = mybir.MatmulPerfMode.DoubleRow
    COPY = mybir.ActivationFunctionType.Copy

    nc = bacc.Bacc(
        "TRN2", target_bir_lowering=False, debug=False, num_devices=N_CORES
    )
    xtl_d = nc.dram_tensor("xtl", [N_PAIR, 128, KT, 256], f16,
                           kind="ExternalInput")
    wtl_d = nc.dram_tensor("wtl", [128, KT, O_SH], f16, kind="ExternalInput")
    a_d = nc.dram_tensor("a", [R, K], f16, kind="ExternalInput")
    bt_d = nc.dram_tensor("bt", [R, O_SH], f16, kind="ExternalInput")
    bias_d = nc.dram_tensor("bias", [O_SH], f32, kind="ExternalInput")
    out_d = nc.dram_tensor("out", [M_SH, O_SH], f32, kind="ExternalOutput")

    with tile.TileContext(nc) as tc:
        with (
            tc.tile_pool(name="const", bufs=1) as const,
            tc.tile_pool(name="wfp", bufs=1) as wfp,
            tc.tile_pool(name="wsp", bufs=2) as wsp,
            tc.tile_pool(name="wp8p", bufs=1) as wp8p,
            tc.tile_pool(name="xfp", bufs=3) as xfp,
            tc.tile_pool(name="xpp", bufs=3) as xpp,
            tc.tile_pool(name="outp", bufs=3) as outp,
        ):
            # ---- constants ----
            junk = const.tile([128, 512], f16)
            nc.vector.memset(junk[:], 0.0)
            # small consts ride HWDGE (sync) ahead of the W chunks so the
            # BA stream can start ~8us in; gpsimd SWDGE would land ~17us.
            a_sb = const.tile([R, K], f16)
            nc.sync.dma_start(a_sb[:], a_d[:])
            bt_sb = const.tile([R, O_SH], f16)
            nc.sync.dma_start(bt_sb[:], bt_d[:])
            bt2 = const.tile([R, O_SH], f16)  # 2 * B^T (ACT engine is idle)
            nc.scalar.activation(bt2[:], bt_sb[:],
                                 mybir.ActivationFunctionType.Copy,
                                 scale=SCALING)
            bias_bc = const.tile([128, O_SH], f32)
            nc.gpsimd.dma_start(bias_bc[:], bias_d[:].partition_broadcast(128))

            wf = {}       # folded W tiles, f16 [128, O_SH]
            wp8 = [wp8p.tile([128, 2, O_SH], f8, name=f"wp8_{kp}")
                   for kp in range(N_KP)]

            def ba_fold(kt, pba_pool, w16):
                """BA matmul + fold (w16 + 2BA^T) -> wf[kt] (f16)."""
                ks = slice(kt * 128, (kt + 1) * 128)
                wft = wfp.tile([128, O_SH], f16, name=f"wf{kt}")
                for h in range(2):
                    hs = slice(h * 512, (h + 1) * 512)
                    pba = pba_pool.tile([128, 512], f32, name=f"pba{h}")
                    nc.tensor.matmul(
                        pba[:], a_sb[:, ks], bt2[:, hs], start=True, stop=True
                    )
                    nc.vector.tensor_add(wft[:, hs], pba[:], w16[:, hs])
                wf[kt] = wft

            xfs = [None] * N_PAIR  # f16 x tiles [128, KT, 256] per m-pair
            xps = [None] * N_PAIR  # fp8 paired x tiles [128, N_KP, 2, 256]

            def fetch_x(pr):
                xf = xfp.tile([128, KT, 256], f16, name="xf")
                nc.scalar.dma_start(xf[:], xtl_d[pr, :, :, :])
                xp = xpp.tile([128, N_KP, 2, 256], f8, name="xp")
                for kp in range(N_KP):
                    for i in range(2):
                        nc.scalar.activation(
                            xp[:, kp, i, :], xf[:, 2 * kp + i, :],
                            COPY, scale=1.0 / WS8,
                        )
                xfs[pr], xps[pr] = xf, xp

            def xslice(mt, kt):
                pr, half = divmod(mt, 2)
                return xfs[pr][:, kt, half * 128 : half * 128 + 128]

            def xslice8(mt, kp):
                pr, half = divmod(mt, 2)
                return xps[pr][:, kp, :, half * 128 : half * 128 + 128]

            def dr_pass(mt, oc, pm, kp, start=False):
                ocs = slice(oc * 512, (oc + 1) * 512)
                nc.tensor.matmul(
                    pm[:], xslice8(mt, kp), wp8[kp][:, :, ocs],
                    start=start, stop=False, perf_mode=DR,
                )

            def f16_pass(mt, oc, pm, kt):
                nc.tensor.matmul(
                    pm[:], xslice(mt, kt), wf[kt][:, oc * 512 : (oc + 1) * 512],
                    start=False, stop=False,
                )

            def close_group(mt, oc, pm, osb):
                nc.tensor.matmul(
                    pm[:], xslice(mt, KT - 1),
                    wf[KT - 1][:, oc * 512 : (oc + 1) * 512],
                    start=False, stop=True,
                )
                hs = slice(oc * 512, (oc + 1) * 512)
                nc.vector.tensor_add(osb[:, hs], pm[:], bias_bc[:, hs])

            HEAD_MTS = (0, 1, 2)

            with tc.tile_pool(name="bap", bufs=1, space="PSUM") as bap, \
                 tc.tile_pool(name="hps", bufs=1, space="PSUM") as hps:
                # warmup burst flips the PE clock gate early and bridges the
                # gap until the BA stream's inputs land (~8us)
                pwarm = bap.tile([128, 512], f32, name="pba0")
                for _ in range(14):
                    nc.tensor.matmul(
                        pwarm[:], junk[:, 0:128], junk[:], start=True, stop=True
                    )

                fetch_x(0)
                fetch_x(1)

                hpm = {
                    (mt, oc): hps.tile([128, 512], f32, name=f"h{mt}_{oc}")
                    for mt in HEAD_MTS for oc in range(2)
                }

                # W chunk stream: BA+fold per kt; chunk 0 also feeds the fp8
                # quantize; chunks 1..3 are absorbed k-outer by head groups.
                for c in range(N_WCH):
                    w16c = wsp.tile([128, 8, O_SH], f16, name="w16c")
                    nc.sync.dma_start(w16c[:], wtl_d[:, c * 8 : (c + 1) * 8, :])
                    for j in range(8):
                        ba_fold(c * 8 + j, bap, w16c[:, j, :])
                    if c == 0:
                        for kp in range(N_KP):
                            for i in range(2):
                                nc.scalar.activation(
                                    wp8[kp][:, i, :], wf[2 * kp + i][:],
                                    COPY, scale=WS8,
                                )
                        # open head groups with fp8 DR passes
                        for mt in HEAD_MTS:
                            for oc in range(2):
                                dr_pass(mt, oc, hpm[(mt, oc)], 0, start=True)
                        for kp in range(1, N_KP):
                            for mt in HEAD_MTS:
                                for oc in range(2):
                                    dr_pass(mt, oc, hpm[(mt, oc)], kp)
                    else:
                        for j in range(8):
                            kt = c * 8 + j
                            if kt == KT - 1:
                                continue  # kt31 carried by close_group
                            for mt in HEAD_MTS:
                                for oc in range(2):
                                    f16_pass(mt, oc, hpm[(mt, oc)], kt)

                fetch_x(2)
                osbh = {}
                for mt in HEAD_MTS:
                    osbh[mt] = outp.tile([128, O_SH], f32, name="osb")
                    for oc in range(2):
                        close_group(mt, oc, hpm[(mt, oc)], osbh[mt])
                    nc.sync.dma_start(
                        out_d[mt * 128 : (mt + 1) * 128, :], osbh[mt][:]
                    )

            # ---- steady phase ----
            with tc.tile_pool(name="sps", bufs=4, space="PSUM") as sps:
                for mt in range(len(HEAD_MTS), N_MT):
                    pr = mt // 2
                    if mt % 2 == 0 and pr + 1 < N_PAIR:
                        fetch_x(pr + 1)
                    osb = outp.tile([128, O_SH], f32, name="osb")
                    for oc in range(2):
                        pm = sps.tile([128, 512], f32, name="pm")
                        dr_pass(mt, oc, pm, 0, start=True)
                        for kp in range(1, N_KP):
                            dr_pass(mt, oc, pm, kp)
                        for kt in range(KT16_0, KT - 1):
                            f16_pass(mt, oc, pm, kt)
                        close_group(mt, oc, pm, osb)
                    nc.sync.dma_start(
                        out_d[mt * 128 : (mt + 1) * 128, :], osb[:]
                    )

    nc.compile()
    return nc


def _get_nc():
    if "nc" not in _NC_CACHE:
        _NC_CACHE["nc"] = _build()
    return _NC_CACHE["nc"]


def kernel(x, weight, bias, A, B):
    global LAST_RESULT
    from concourse.bass_utils import run_bass_kernel_spmd

    x = np.asarray(x, dtype=np.float32).reshape(M, K)
    weight = np.asarray(weight, dtype=np.float32)
    bias = np.asarray(bias, dtype=np.float32)
    A = np.asarray(A, dtype=np.float32)
    B = np.asarray(B, dtype=np.float32)

    # Host-side layout prep (transposes + f16 casts only; no arithmetic).
    xtl_slabs = []
    for mi in range(M_SPLIT):
        xt = x[mi * M_SH : (mi + 1) * M_SH].T  # [K, M_SH]
        v = xt.reshape(KT, 128, N_PAIR, 256).transpose(2, 1, 0, 3)
        xtl_slabs.append(np.ascontiguousarray(v, dtype=np.float16))
    wtl_slabs, bt_slabs, bias_slabs = [], [], []
    for oi in range(O_SPLIT):
        os_ = slice(oi * O_SH, (oi + 1) * O_SH)
        wt = weight[os_].T  # [K, O_SH]
        v = wt.reshape(KT, 128, O_SH).transpose(1, 0, 2)
        wtl_slabs.append(np.ascontiguousarray(v, dtype=np.float16))
        bt_slabs.append(np.ascontiguousarray(B[os_].T, dtype=np.float16))
        bias_slabs.append(np.ascontiguousarray(bias[os_]))
    a_f16 = np.ascontiguousarray(A, dtype=np.float16)

    nc = _get_nc()
    in_maps = []
    for c in range(N_CORES):
        mi, oi = divmod(c, O_SPLIT)
        in_maps.append(
            {
                "xtl": xtl_slabs[mi],
                "wtl": wtl_slabs[oi],
                "a": a_f16,
                "bt": bt_slabs[oi],
                "bias": bias_slabs[oi],
            }
        )

    res = run_bass_kernel_spmd(nc, in_maps, list(range(N_CORES)))
    LAST_RESULT = res

    out = np.empty((M, OUT_F), np.float32)
    for c in range(N_CORES):
        mi, oi = divmod(c, O_SPLIT)
        out[mi * M_SH : (mi + 1) * M_SH, oi * O_SH : (oi + 1) * O_SH] = (
            res.results[c]["out"]
        )
    return out.reshape(4, 2048, OUT_F)
